# revision 3
# baseline (speedup 1.0000x reference)
"""MeshUnPool gather kernel for 8 Trainium2 NeuronCores.

reference: out[i, :] = features[parent_idx[i], :]
  features: [500000, 256] f32 (512 MB), parent_idx: [1000000] int64/int32,
  out: [1000000, 256] f32 (1 GB).

Sharding: output rows are sharded across the 8 cores; the feature table is
replicated. Each core gathers its 125952 rows (0.76% pad) with indirect
row-gather DMAs (128 rows per instruction — the HW DGE consumes one index
per SBUF partition) and stores contiguously.
"""

import numpy as np

import concourse.bass as bass
import concourse.bacc as bacc
import concourse.mybir as mybir
import concourse.tile as tile
from concourse.bass_utils import run_bass_kernel_spmd

N_POOLED = 500000
N_UNPOOLED = 1000000
C = 256
NCORES = 8
P = 128

# rows per core = P * GPB * NB ; 8 * 125952 = 1007616 (0.76% pad over 1e6)
GPB = 24          # gathers (128 rows each) per store block
NB = 41           # store blocks per core
ROWS_PER_CORE = P * GPB * NB

_cache = {}


def _build():
    nc = bacc.Bacc("TRN2", target_bir_lowering=False, debug=False,
                   num_devices=NCORES)
    feat = nc.dram_tensor("features", [N_POOLED, C], mybir.dt.float32,
                          kind="ExternalInput").ap()
    # host ships idx pre-wrapped: element (p, t) = parent_idx[t*128 + p]
    idx = nc.dram_tensor("parent_idx", [P, GPB * NB], mybir.dt.int32,
                         kind="ExternalInput").ap()
    out = nc.dram_tensor("out", [ROWS_PER_CORE, C], mybir.dt.float32,
                         kind="ExternalOutput").ap()

    with tile.TileContext(nc) as tc:
        with tc.tile_pool(name="g", bufs=3) as gp, \
             tc.tile_pool(name="i", bufs=1) as ip:
            idx_tile = ip.tile([P, GPB * NB], mybir.dt.int32)
            nc.scalar.dma_start(out=idx_tile[:], in_=idx[:])
            for b in range(NB):
                gtile = gp.tile([P, GPB * C], mybir.dt.float32)
                for j in range(GPB):
                    t = b * GPB + j
                    nc.gpsimd.indirect_dma_start(
                        out=gtile[:, j * C:(j + 1) * C],
                        out_offset=None,
                        in_=feat[:],
                        in_offset=bass.IndirectOffsetOnAxis(
                            ap=idx_tile[:, t:t + 1], axis=0),
                    )
                # rows of block b: row (t*128 + p) = gtile[p, j*C:(j+1)*C]
                nc.sync.dma_start(
                    out=out[b * GPB * P:(b + 1) * GPB * P, :].rearrange(
                        "(j p) c -> p j c", p=P),
                    in_=gtile[:].rearrange("p (j c) -> p j c", c=C),
                )
    nc.compile()
    return nc


def _run(features, parent_idx, **spmd_kwargs):
    feat = np.ascontiguousarray(np.asarray(features), dtype=np.float32)
    idx32 = np.zeros(ROWS_PER_CORE * NCORES, dtype=np.int32)
    idx32[:N_UNPOOLED] = np.asarray(parent_idx).astype(np.int32)
    # per core: wrap [ROWS] -> [P, T] with (p, t) = idx[t*128 + p]
    shards = idx32.reshape(NCORES, GPB * NB, P).transpose(0, 2, 1)

    if "nc" not in _cache:
        _cache["nc"] = _build()
    nc = _cache["nc"]

    in_maps = [{"features": feat,
                "parent_idx": np.ascontiguousarray(shards[c])}
               for c in range(NCORES)]
    res = run_bass_kernel_spmd(nc, in_maps, core_ids=list(range(NCORES)),
                               **spmd_kwargs)
    out = np.concatenate([r["out"] for r in res.results], axis=0)[:N_UNPOOLED]
    return out, res


def kernel(features, parent_idx):
    out, _ = _run(features, parent_idx)
    return out
